# revision 33
# baseline (speedup 1.0000x reference)
"""BitLinear + tanh head, 8-way batch-parallel on one TRN2 chip; the weighted
cumsum + phase wrap run on the host (f64 cumsum, f32 wrap) where they are
essentially free and numerically closest to the f32 reference.

Math (per batch element, matching the BitNet b1.58 reference forward pass):
  amax_t  = max(max_d |x[t,d]|, 1e-5)
  xi[t,d] = rne(x[t,d] * 127/amax_t)            # ints in [-127,127]
  mw      = max(mean|W|, 1e-5)
  wi[o,d] = clip(rne(W[o,d]/mw), -1, 1)         # ternary ints
  I[t,o]  = sum_d xi[t,d]*wi[o,d]               # EXACT int matmul, f32 PSUM
  v[t,o]  = tanh(I * (amax_t/127 * mw) + b[o])  # device output [T, O]
  host:     phase = wrap(cumsum_t(v) * pi * cumsum_weight)

The matmul runs "flipped": PSUM is [t, o] with the quantized-x tile as the
stationary operand, so the per-token descale amax_t/127*mw is a per-PARTITION
scalar fused directly into the tanh activation (no separate multiply pass,
no broadcast of the scale over partitions, no on-device scan).

All rounding uses the fp32 magic constant 1.5*2**23 (single f32 rne to the
integer grid, bit-matching the reference); quantized ints live in bf16
(exact for |int| <= 256). Transposes go through the DMA xbar. x rows are
loaded four 128-row tiles per DMA; amax runs as one grouped reduce.
"""

import os
import sys

for _p in ("/opt/trn_rl_repo", "/root/.axon_site/_ro/trn_rl_repo"):
    if os.path.isdir(_p) and _p not in sys.path:
        sys.path.insert(0, _p)

import numpy as np
from contextlib import ExitStack

import concourse.bass as bass
from concourse import bacc
from concourse import mybir
from concourse.bass_utils import run_bass_kernel_spmd
from concourse.tile import TileContext

F32 = mybir.dt.float32
BF16 = mybir.dt.bfloat16
MAGIC = 12582912.0  # 1.5 * 2**23, fp32 round-to-nearest-even trick
PI = float(np.pi)
N_CORES = 8
Alu = mybir.AluOpType
Act = mybir.ActivationFunctionType

# Engine notes (hardware-verified on this problem):
#  - PSUM-reading vector ops and tensor_tensor_scan are DVE-only.
#  - GpSimd TENSOR_SCALAR / dma accum are software-emulated (14us/tile) - avoid
#    for compute; GpSimd *can* cheaply issue plain DMAs (software DGE).
#  - DVE TENSOR_SCALAR on all-SBUF f32 runs in 2x mode (~0.6ns/elem).
#  - PE transposes are replaced by DMA-xbar transposes (2-byte dtypes only).
GROUP = 4          # x-tiles per load/amax group
LOOKAHEAD_G = 2    # groups quantized ahead of the matmul stream


def build(T: int = 4096, D: int = 1024, O: int = 1024, b_nonzero: bool = False):
    """Per-core Bass program. Output: v = tanh(...) in [T, O] f32."""
    NTT = T // 128
    NO = O // 128
    NK = D // 128
    NOB = O // 512      # 512-col psum banks across o
    NG = NTT // GROUP   # x groups

    nc = bacc.Bacc("TRN2", target_bir_lowering=False, debug=False)
    x_d = nc.dram_tensor("x", [T, D], F32, kind="ExternalInput")
    w_d = nc.dram_tensor("W", [O, D], F32, kind="ExternalInput")
    b_d = nc.dram_tensor("b", [O], F32, kind="ExternalInput")
    out_d = nc.dram_tensor("out_t", [T, O], F32, kind="ExternalOutput")

    with TileContext(nc) as tc, ExitStack() as ctx:
        ep = ctx.enter_context

        consts = ep(tc.tile_pool(name="consts", bufs=1))
        wpool = ep(tc.tile_pool(name="wpool", bufs=4))
        rwpool = ep(tc.tile_pool(name="rwpool", bufs=1))
        wqpool = ep(tc.tile_pool(name="wqpool", bufs=1))
        qpool = ep(tc.tile_pool(name="qpool", bufs=1))
        xgpool = ep(tc.tile_pool(name="xgpool", bufs=2))
        rpool = ep(tc.tile_pool(name="rpool", bufs=2))
        hpool = ep(tc.tile_pool(name="hpool", bufs=3))
        smpool = ep(tc.tile_pool(name="smpool", bufs=4))
        vpool = ep(tc.tile_pool(name="vpool", bufs=3))
        mm_ps = ep(tc.tile_pool(name="mm_ps", bufs=6, space="PSUM"))
        mi_ps = ep(tc.tile_pool(name="mi_ps", bufs=2, space="PSUM"))

        # ---------------- constants ----------------
        magic = consts.tile([128, 1], F32)
        nc.vector.memset(magic[:], MAGIC)
        ones_col = consts.tile([128, 1], F32)
        nc.vector.memset(ones_col[:], 1.0)
        ones_row = consts.tile([1, 128], F32)
        nc.vector.memset(ones_row[:], 1.0)

        # ---------------- weight quant (2 o-tiles per quarter) ----------------
        # mean|W| first: stream quarters, grouped abs row-sums.
        NWQ = NO // 2
        wgs = []
        asum = consts.tile([128, NO], F32)
        for q in range(NWQ):
            wg = wpool.tile([128, 2, D], F32, tag="wload")
            nc.sync.dma_start(
                out=wg[:], in_=w_d[q * 256 : (q + 1) * 256, :].rearrange(
                    "(s p) d -> p s d", p=128))
            wgs.append(wg)
            if q == 0:
                # prefetch the first x group between W quarters so its
                # quantization overlaps the W scale computation
                xg0 = xgpool.tile([128, GROUP, D], F32, tag="xg", name="xg")
                nc.sync.dma_start(
                    out=xg0[:],
                    in_=x_d[0 : GROUP * 128, :].rearrange(
                        "(s p) d -> p s d", p=128))
        for q in range(NWQ):
            nc.vector.tensor_reduce(
                out=asum[:, q * 2 : q * 2 + 2], in_=wgs[q][:],
                axis=mybir.AxisListType.X, op=Alu.add,
                apply_absolute_value=True)
        asum1 = consts.tile([128, 1], F32)
        nc.vector.tensor_reduce(
            out=asum1[:], in_=asum[:], axis=mybir.AxisListType.X, op=Alu.add)
        tot_ps = mi_ps.tile([1, 1], F32, tag="misc")
        nc.tensor.matmul(tot_ps[:], lhsT=asum1[:], rhs=ones_col[:],
                         start=True, stop=True)
        ms = consts.tile([1, 2], F32)
        nc.vector.tensor_scalar(out=ms[:, 0:1], in0=tot_ps[:],
                                scalar1=1.0 / float(O * D), scalar2=1e-5,
                                op0=Alu.mult, op1=Alu.max)
        nc.vector.reciprocal(out=ms[:, 1:2], in_=ms[:, 0:1])
        bc_ps = mi_ps.tile([128, 2], F32, tag="misc")
        nc.tensor.matmul(bc_ps[:], lhsT=ones_row[:], rhs=ms[:],
                         start=True, stop=True)
        msb = consts.tile([128, 2], F32)
        nc.vector.tensor_copy(out=msb[:], in_=bc_ps[:])
        mean_b = msb[:, 0:1]  # mw broadcast over partitions
        sw_b = msb[:, 1:2]    # 1/mw broadcast

        wqt = qpool.tile([128, NO, NK, 128], BF16, tag="wqt")
        xqt = qpool.tile([128, NTT, NK, 128], BF16, tag="xqt")

        am127 = consts.tile([128, NTT], F32)   # amax'/127 per token
        rall = consts.tile([128, NTT], F32)    # 127/amax' per token
        ammw = consts.tile([128, NTT], F32)    # amax'/127 * mw (tanh scale)

        if b_nonzero:
            from concourse.masks import make_identity
            ident = consts.tile([128, 128], F32)
            make_identity(nc, ident[:])
            b_row = consts.tile([1, O], F32)
            nc.sync.dma_start(
                out=b_row[:], in_=b_d[:].rearrange("(one o) -> one o", one=1))
            rsw = consts.tile([128, NTT], F32)   # rall * (1/mw) per token
            rsw_row = consts.tile([1, T], F32)   # transposed to a row

        # quantize W quarters: batched rne+clip (the scale sw is global)
        for q in range(NWQ):
            wg = wgs[q]
            rwg = rwpool.tile([128, 2, D], F32, tag="rw", name="rw")
            nc.scalar.activation(out=rwg[:], in_=wg[:], func=Act.Identity,
                                 bias=magic[:], scale=sw_b)
            nc.vector.tensor_scalar(out=wg[:], in0=rwg[:], scalar1=MAGIC,
                                    scalar2=1.0, op0=Alu.subtract, op1=Alu.min)
            wqg = wqpool.tile([128, 2, D], BF16, tag="wq", name="wq")
            nc.vector.tensor_scalar(out=wqg[:], in0=wg[:], scalar1=-1.0,
                                    scalar2=None, op0=Alu.max)
            for j in range(2):
                m = q * 2 + j
                nc.sync.dma_start_transpose(out=wqt[:, m, :, :],
                                            in_=wqg[:, j, :])

        def quant_group(g, xg=None):
            """Load 4 x row-tiles in one DMA; quantize + xbar each."""
            if xg is None:
                xg = xgpool.tile([128, GROUP, D], F32, tag="xg", name="xg")
                nc.sync.dma_start(
                    out=xg[:],
                    in_=x_d[g * GROUP * 128 : (g + 1) * GROUP * 128, :]
                    .rearrange("(s p) d -> p s d", p=128))
            amg = smpool.tile([128, GROUP], F32, tag="amg", name="amg")
            nc.vector.tensor_reduce(
                out=amg[:], in_=xg[:], axis=mybir.AxisListType.X,
                op=Alu.max, apply_absolute_value=True)
            c0 = g * GROUP
            nc.vector.tensor_scalar(
                out=am127[:, c0 : c0 + GROUP], in0=amg[:], scalar1=1e-5,
                scalar2=1.0 / 127.0, op0=Alu.max, op1=Alu.mult)
            nc.vector.reciprocal(out=rall[:, c0 : c0 + GROUP],
                                 in_=am127[:, c0 : c0 + GROUP])
            nc.vector.tensor_scalar(
                out=ammw[:, c0 : c0 + GROUP], in0=am127[:, c0 : c0 + GROUP],
                scalar1=mean_b, scalar2=None, op0=Alu.mult)
            if b_nonzero:
                nc.vector.tensor_scalar(
                    out=rsw[:, c0 : c0 + GROUP], in0=rall[:, c0 : c0 + GROUP],
                    scalar1=sw_b, scalar2=None, op0=Alu.mult)
            for j in range(GROUP):
                tt = c0 + j
                r_t = rpool.tile([128, D], F32, tag="r", name="r_t")
                nc.vector.tensor_scalar(out=r_t[:], in0=xg[:, j, :],
                                        scalar1=rall[:, tt : tt + 1],
                                        scalar2=MAGIC, op0=Alu.mult,
                                        op1=Alu.add)
                h = hpool.tile([128, D], BF16, tag="h", name="h")
                nc.vector.tensor_scalar(out=h[:], in0=r_t[:], scalar1=MAGIC,
                                        scalar2=None, op0=Alu.subtract)
                nc.sync.dma_start_transpose(out=xqt[:, tt, :, :], in_=h[:])
                if b_nonzero:
                    rp = mi_ps.tile([1, 128], F32, tag="misc", name="rp")
                    nc.tensor.transpose(rp[:], rsw[:, tt : tt + 1], ident[:])
                    nc.scalar.copy(
                        out=rsw_row[0:1, tt * 128 : (tt + 1) * 128], in_=rp[:])

        def mm_tile(tt):
            """I[t-block, :] matmul + fused descale/tanh + store."""
            psums = [mm_ps.tile([128, 512], F32, tag="mm", name="mm")
                     for _ in range(NOB)]
            for k in range(NK):
                for oi, P in enumerate(psums):
                    nc.tensor.matmul(
                        P[:], lhsT=xqt[:, tt, k, :],
                        rhs=wqt[:, oi * 4 : (oi + 1) * 4, k, :],
                        start=(k == 0), stop=(k == NK - 1 and not b_nonzero))
            if b_nonzero:
                for oi, P in enumerate(psums):
                    nc.tensor.matmul(
                        P[:],
                        lhsT=rsw_row[0:1, tt * 128 : (tt + 1) * 128],
                        rhs=b_row[0:1, oi * 512 : (oi + 1) * 512],
                        start=False, stop=True)
            v = vpool.tile([128, O], F32, tag="v", name="v")
            for oi, P in enumerate(psums):
                nc.scalar.activation(
                    out=v[:, oi * 512 : (oi + 1) * 512], in_=P[:],
                    func=Act.Tanh, bias=0.0, scale=ammw[:, tt : tt + 1])
            nc.scalar.dma_start(
                out=out_d[tt * 128 : (tt + 1) * 128, :], in_=v[:])

        # ---------------- schedule ----------------
        quant_group(0, xg=xg0)
        for g in range(1, LOOKAHEAD_G):
            quant_group(g)
        for tt in range(NTT):
            if tt % GROUP == 0 and tt // GROUP + LOOKAHEAD_G < NG:
                quant_group(tt // GROUP + LOOKAHEAD_G)
            mm_tile(tt)

    nc.finalize()
    return nc


def kernel(x: np.ndarray, W: np.ndarray, b: np.ndarray,
           cumsum_weight: np.ndarray) -> np.ndarray:
    B, T, D = x.shape
    O = W.shape[0]
    assert B == N_CORES
    cw = float(np.asarray(cumsum_weight).reshape(-1)[0])
    if cw == 0.0:
        # phase is identically 0; wrap(0) = 0
        return np.zeros((B, T, O), dtype=np.float32)
    b = np.ascontiguousarray(np.asarray(b, dtype=np.float32))
    nc = build(T=T, D=D, O=O, b_nonzero=bool(np.any(b != 0.0)))
    x = np.ascontiguousarray(np.asarray(x, dtype=np.float32))
    W = np.ascontiguousarray(np.asarray(W, dtype=np.float32))
    in_maps = [{"x": x[i], "W": W, "b": b} for i in range(N_CORES)]
    res = run_bass_kernel_spmd(nc, in_maps, list(range(N_CORES)))
    return postprocess([res.results[i]["out_t"] for i in range(N_CORES)], cw)


def postprocess(v_list, cw: float) -> np.ndarray:
    """Device gives v = tanh(...) in [T, O]. Host: S = cumsum_t(v) in f64
    (closest to any decent f32 cumsum), phase = f32(S*c), then wrap to
    (-pi, pi] with the reference's own f32 ops."""
    pi32 = np.float32(np.pi)
    two_pi = np.float32(2.0 * float(np.float32(np.pi)))
    c = np.float64(PI * cw)
    outs = []
    for v in v_list:
        S = np.cumsum(np.asarray(v, dtype=np.float64), axis=0)
        phase = (S * c).astype(np.float32)
        phase = np.remainder(phase + pi32, two_pi) - pi32
        outs.append(phase)
    return np.stack(outs, axis=0)


# revision 35
# speedup vs baseline: 1.0797x; 1.0797x over previous
"""BitLinear + tanh head, 8-way batch-parallel on one TRN2 chip; the weighted
cumsum + phase wrap run on the host (f64 cumsum, f32 wrap) where they are
essentially free and numerically closest to the f32 reference.

Math (per batch element, matching the BitNet b1.58 reference forward pass):
  amax_t  = max(max_d |x[t,d]|, 1e-5)
  xi[t,d] = rne(x[t,d] * 127/amax_t)            # ints in [-127,127]
  mw      = max(mean|W|, 1e-5)
  wi[o,d] = clip(rne(W[o,d]/mw), -1, 1)         # ternary ints
  I[t,o]  = sum_d xi[t,d]*wi[o,d]               # EXACT int matmul, f32 PSUM
  v[t,o]  = tanh(I * (amax_t/127 * mw) + b[o])  # device output [T, O]
  host:     phase = wrap(cumsum_t(v) * pi * cumsum_weight)

The matmul runs "flipped": PSUM is [t, o] with the quantized-x tile as the
stationary operand, so the per-token descale amax_t/127*mw is a per-PARTITION
scalar fused directly into the tanh activation (no separate multiply pass,
no broadcast of the scale over partitions, no on-device scan).

All rounding uses the fp32 magic constant 1.5*2**23 (single f32 rne to the
integer grid, bit-matching the reference); quantized ints live in bf16
(exact for |int| <= 256). Transposes go through the DMA xbar. x rows are
loaded four 128-row tiles per DMA; amax runs as one grouped reduce.
"""

import os
import sys

for _p in ("/opt/trn_rl_repo", "/root/.axon_site/_ro/trn_rl_repo"):
    if os.path.isdir(_p) and _p not in sys.path:
        sys.path.insert(0, _p)

import numpy as np
from contextlib import ExitStack

import concourse.bass as bass
from concourse import bacc
from concourse import mybir
from concourse.bass_utils import run_bass_kernel_spmd
from concourse.tile import TileContext

F32 = mybir.dt.float32
BF16 = mybir.dt.bfloat16
MAGIC = 12582912.0  # 1.5 * 2**23, fp32 round-to-nearest-even trick
PI = float(np.pi)
N_CORES = 8
Alu = mybir.AluOpType
Act = mybir.ActivationFunctionType

# Engine notes (hardware-verified on this problem):
#  - PSUM-reading vector ops and tensor_tensor_scan are DVE-only.
#  - GpSimd TENSOR_SCALAR / dma accum are software-emulated (14us/tile) - avoid
#    for compute; GpSimd *can* cheaply issue plain DMAs (software DGE).
#  - DVE TENSOR_SCALAR on all-SBUF f32 runs in 2x mode (~0.6ns/elem).
#  - PE transposes are replaced by DMA-xbar transposes (2-byte dtypes only).
GROUP = 4          # x-tiles per load/amax group
LOOKAHEAD_G = 2    # groups quantized ahead of the matmul stream


def build(T: int = 4096, D: int = 1024, O: int = 1024, b_nonzero: bool = False):
    """Per-core Bass program. Output: v = tanh(...) in [T, O] f32."""
    NTT = T // 128
    NO = O // 128
    NK = D // 128
    NOB = O // 512      # 512-col psum banks across o
    NG = NTT // GROUP   # x groups

    nc = bacc.Bacc("TRN2", target_bir_lowering=False, debug=False)
    x_d = nc.dram_tensor("x", [T, D], F32, kind="ExternalInput")
    w_d = nc.dram_tensor("W", [O, D], F32, kind="ExternalInput")
    b_d = nc.dram_tensor("b", [O], F32, kind="ExternalInput")
    out_d = nc.dram_tensor("out_t", [T, O], F32, kind="ExternalOutput")

    with TileContext(nc) as tc, ExitStack() as ctx:
        ep = ctx.enter_context

        consts = ep(tc.tile_pool(name="consts", bufs=1))
        wpool = ep(tc.tile_pool(name="wpool", bufs=4))
        rwpool = ep(tc.tile_pool(name="rwpool", bufs=1))
        wqpool = ep(tc.tile_pool(name="wqpool", bufs=1))
        qpool = ep(tc.tile_pool(name="qpool", bufs=1))
        xgpool = ep(tc.tile_pool(name="xgpool", bufs=2))
        rpool = ep(tc.tile_pool(name="rpool", bufs=1))
        hpool = ep(tc.tile_pool(name="hpool", bufs=2))
        smpool = ep(tc.tile_pool(name="smpool", bufs=4))
        vpool = ep(tc.tile_pool(name="vpool", bufs=2))
        mm_ps = ep(tc.tile_pool(name="mm_ps", bufs=6, space="PSUM"))
        mi_ps = ep(tc.tile_pool(name="mi_ps", bufs=2, space="PSUM"))

        # ---------------- constants ----------------
        magic = consts.tile([128, 1], F32)
        nc.vector.memset(magic[:], MAGIC)
        ones_col = consts.tile([128, 1], F32)
        nc.vector.memset(ones_col[:], 1.0)
        ones_row = consts.tile([1, 128], F32)
        nc.vector.memset(ones_row[:], 1.0)

        # ---------------- weight quant (2 o-tiles per quarter) ----------------
        # mean|W| first: stream quarters, grouped abs row-sums.
        NWQ = NO // 2
        wgs = []
        asum = consts.tile([128, NO], F32)
        for q in range(NWQ):
            wg = wpool.tile([128, 2, D], F32, tag="wload")
            nc.sync.dma_start(
                out=wg[:], in_=w_d[q * 256 : (q + 1) * 256, :].rearrange(
                    "(s p) d -> p s d", p=128))
            wgs.append(wg)
            if q == 0:
                # prefetch the first x group between W quarters so its
                # quantization overlaps the W scale computation
                xg0 = xgpool.tile([128, GROUP, D], F32, tag="xg", name="xg")
                nc.sync.dma_start(
                    out=xg0[:],
                    in_=x_d[0 : GROUP * 128, :].rearrange(
                        "(s p) d -> p s d", p=128))
        for q in range(NWQ):
            nc.vector.tensor_reduce(
                out=asum[:, q * 2 : q * 2 + 2], in_=wgs[q][:],
                axis=mybir.AxisListType.X, op=Alu.add,
                apply_absolute_value=True)
        asum1 = consts.tile([128, 1], F32)
        nc.vector.tensor_reduce(
            out=asum1[:], in_=asum[:], axis=mybir.AxisListType.X, op=Alu.add)
        tot_ps = mi_ps.tile([1, 1], F32, tag="misc")
        nc.tensor.matmul(tot_ps[:], lhsT=asum1[:], rhs=ones_col[:],
                         start=True, stop=True)
        ms = consts.tile([1, 2], F32)
        nc.vector.tensor_scalar(out=ms[:, 0:1], in0=tot_ps[:],
                                scalar1=1.0 / float(O * D), scalar2=1e-5,
                                op0=Alu.mult, op1=Alu.max)
        nc.vector.reciprocal(out=ms[:, 1:2], in_=ms[:, 0:1])
        bc_ps = mi_ps.tile([128, 2], F32, tag="misc")
        nc.tensor.matmul(bc_ps[:], lhsT=ones_row[:], rhs=ms[:],
                         start=True, stop=True)
        msb = consts.tile([128, 2], F32)
        nc.vector.tensor_copy(out=msb[:], in_=bc_ps[:])
        mean_b = msb[:, 0:1]  # mw broadcast over partitions
        sw_b = msb[:, 1:2]    # 1/mw broadcast

        wqt = qpool.tile([128, NO, NK, 128], BF16, tag="wqt")
        xqt = qpool.tile([128, NTT, NK, 128], BF16, tag="xqt")

        am127 = consts.tile([128, NTT], F32)   # amax'/127 per token
        rall = consts.tile([128, NTT], F32)    # 127/amax' per token
        ammw = consts.tile([128, NTT], F32)    # amax'/127 * mw (tanh scale)

        if b_nonzero:
            from concourse.masks import make_identity
            ident = consts.tile([128, 128], F32)
            make_identity(nc, ident[:])
            b_row = consts.tile([1, O], F32)
            nc.sync.dma_start(
                out=b_row[:], in_=b_d[:].rearrange("(one o) -> one o", one=1))
            rsw = consts.tile([128, NTT], F32)   # rall * (1/mw) per token
            rsw_row = consts.tile([1, T], F32)   # transposed to a row

        # quantize W quarters: batched rne+clip (the scale sw is global)
        for q in range(NWQ):
            wg = wgs[q]
            rwg = rwpool.tile([128, 2, D], F32, tag="rw", name="rw")
            nc.scalar.activation(out=rwg[:], in_=wg[:], func=Act.Identity,
                                 bias=magic[:], scale=sw_b)
            nc.vector.tensor_scalar(out=wg[:], in0=rwg[:], scalar1=MAGIC,
                                    scalar2=1.0, op0=Alu.subtract, op1=Alu.min)
            wqg = wqpool.tile([128, 2, D], BF16, tag="wq", name="wq")
            nc.vector.tensor_scalar(out=wqg[:], in0=wg[:], scalar1=-1.0,
                                    scalar2=None, op0=Alu.max)
            nc.sync.dma_start_transpose(
                out=wqt[:, q * 2 : (q + 1) * 2, :, :], in_=wqg[:])

        def quant_group(g, xg=None):
            """Load 4 x row-tiles in one DMA; quantize + xbar each."""
            if xg is None:
                xg = xgpool.tile([128, GROUP, D], F32, tag="xg", name="xg")
                nc.sync.dma_start(
                    out=xg[:],
                    in_=x_d[g * GROUP * 128 : (g + 1) * GROUP * 128, :]
                    .rearrange("(s p) d -> p s d", p=128))
            amg = smpool.tile([128, GROUP], F32, tag="amg", name="amg")
            nc.vector.tensor_reduce(
                out=amg[:], in_=xg[:], axis=mybir.AxisListType.X,
                op=Alu.max, apply_absolute_value=True)
            c0 = g * GROUP
            nc.vector.tensor_scalar(
                out=am127[:, c0 : c0 + GROUP], in0=amg[:], scalar1=1e-5,
                scalar2=1.0 / 127.0, op0=Alu.max, op1=Alu.mult)
            nc.vector.reciprocal(out=rall[:, c0 : c0 + GROUP],
                                 in_=am127[:, c0 : c0 + GROUP])
            nc.vector.tensor_scalar(
                out=ammw[:, c0 : c0 + GROUP], in0=am127[:, c0 : c0 + GROUP],
                scalar1=mean_b, scalar2=None, op0=Alu.mult)
            if b_nonzero:
                nc.vector.tensor_scalar(
                    out=rsw[:, c0 : c0 + GROUP], in0=rall[:, c0 : c0 + GROUP],
                    scalar1=sw_b, scalar2=None, op0=Alu.mult)
            hg = hpool.tile([128, GROUP, D], BF16, tag="h", name="h")
            for j in range(GROUP):
                tt = c0 + j
                r_t = rpool.tile([128, D], F32, tag="r", name="r_t")
                nc.vector.tensor_scalar(out=r_t[:], in0=xg[:, j, :],
                                        scalar1=rall[:, tt : tt + 1],
                                        scalar2=MAGIC, op0=Alu.mult,
                                        op1=Alu.add)
                nc.vector.tensor_scalar(out=hg[:, j, :], in0=r_t[:],
                                        scalar1=MAGIC, scalar2=None,
                                        op0=Alu.subtract)
            nc.sync.dma_start_transpose(
                out=xqt[:, c0 : c0 + GROUP, :, :], in_=hg[:])
            for j in range(GROUP):
                tt = c0 + j
                if b_nonzero:
                    rp = mi_ps.tile([1, 128], F32, tag="misc", name="rp")
                    nc.tensor.transpose(rp[:], rsw[:, tt : tt + 1], ident[:])
                    nc.scalar.copy(
                        out=rsw_row[0:1, tt * 128 : (tt + 1) * 128], in_=rp[:])

        def mm_tile(tt):
            """I[t-block, :] matmul + fused descale/tanh + store."""
            psums = [mm_ps.tile([128, 512], F32, tag="mm", name="mm")
                     for _ in range(NOB)]
            for k in range(NK):
                for oi, P in enumerate(psums):
                    nc.tensor.matmul(
                        P[:], lhsT=xqt[:, tt, k, :],
                        rhs=wqt[:, oi * 4 : (oi + 1) * 4, k, :],
                        start=(k == 0), stop=(k == NK - 1 and not b_nonzero))
            if b_nonzero:
                for oi, P in enumerate(psums):
                    nc.tensor.matmul(
                        P[:],
                        lhsT=rsw_row[0:1, tt * 128 : (tt + 1) * 128],
                        rhs=b_row[0:1, oi * 512 : (oi + 1) * 512],
                        start=False, stop=True)
            v = vpool.tile([128, O], F32, tag="v", name="v")
            for oi, P in enumerate(psums):
                nc.scalar.activation(
                    out=v[:, oi * 512 : (oi + 1) * 512], in_=P[:],
                    func=Act.Tanh, bias=0.0, scale=ammw[:, tt : tt + 1])
            nc.scalar.dma_start(
                out=out_d[tt * 128 : (tt + 1) * 128, :], in_=v[:])

        # ---------------- schedule ----------------
        quant_group(0, xg=xg0)
        for g in range(1, LOOKAHEAD_G):
            quant_group(g)
        for tt in range(NTT):
            if tt % GROUP == 0 and tt // GROUP + LOOKAHEAD_G < NG:
                quant_group(tt // GROUP + LOOKAHEAD_G)
            mm_tile(tt)

    nc.finalize()
    return nc


def kernel(x: np.ndarray, W: np.ndarray, b: np.ndarray,
           cumsum_weight: np.ndarray) -> np.ndarray:
    B, T, D = x.shape
    O = W.shape[0]
    assert B == N_CORES
    cw = float(np.asarray(cumsum_weight).reshape(-1)[0])
    if cw == 0.0:
        # phase is identically 0; wrap(0) = 0
        return np.zeros((B, T, O), dtype=np.float32)
    b = np.ascontiguousarray(np.asarray(b, dtype=np.float32))
    nc = build(T=T, D=D, O=O, b_nonzero=bool(np.any(b != 0.0)))
    x = np.ascontiguousarray(np.asarray(x, dtype=np.float32))
    W = np.ascontiguousarray(np.asarray(W, dtype=np.float32))
    in_maps = [{"x": x[i], "W": W, "b": b} for i in range(N_CORES)]
    res = run_bass_kernel_spmd(nc, in_maps, list(range(N_CORES)))
    return postprocess([res.results[i]["out_t"] for i in range(N_CORES)], cw)


def postprocess(v_list, cw: float) -> np.ndarray:
    """Device gives v = tanh(...) in [T, O]. Host: S = cumsum_t(v) in f64
    (closest to any decent f32 cumsum), phase = f32(S*c), then wrap to
    (-pi, pi] with the reference's own f32 ops."""
    pi32 = np.float32(np.pi)
    two_pi = np.float32(2.0 * float(np.float32(np.pi)))
    c = np.float64(PI * cw)
    outs = []
    for v in v_list:
        S = np.cumsum(np.asarray(v, dtype=np.float64), axis=0)
        phase = (S * c).astype(np.float32)
        phase = np.remainder(phase + pi32, two_pi) - pi32
        outs.append(phase)
    return np.stack(outs, axis=0)
